# revision 16
# baseline (speedup 1.0000x reference)
"""Trainium2 Bass kernel for single-head dense attention without softmax.

Reference computation (B=4, S=4096, H=1024, fp32):
    q    = x @ W^T               [B, S, H]
    attn = (q @ x^T) @ x         [B, S, H]

There is no softmax, so the computation reorders to
    attn[b] = x[b] @ (W^T @ (x[b]^T @ x[b]))
which drops the FLOP count from ~309 GF to ~77 GF total.

Sharding over 8 NeuronCores: core c handles batch b = c//2 and output
columns jcols = [512*j, 512*j+512) with j = c%2.  Each core computes
    G = x[b]^T x[b]  restricted to columns jcols       (pass 1)
    C = W^T G[:, jcols]                                (pass 2)
    out[:, jcols] = x[b] @ C                           (pass 3)
To keep the device program identical across cores (SPMD), the host
permutes the H columns of x (and the H rows of W) per core so the
core's jcols always land in columns [0, 512).  Pass 3 consumes a
host-side transpose of x (fp32 has no DMA-transpose path on TRN2).
"""

import numpy as np

import concourse.bass as bass
import concourse.mybir as mybir
import concourse.tile as tile
from concourse import bacc
from concourse.bass_utils import run_bass_kernel_spmd

P = 128          # partitions / matmul contraction tile
S = 4096         # sequence length
H = 1024         # hidden
NJ = 512         # output columns per core
KS = S // P      # 32 sequence tiles
KH = H // P      # 8 hidden tiles
N_CORES = 8

F32R = mybir.dt.float32r
F32 = mybir.dt.float32

_CACHE: dict = {}


def build_kernel():
    nc = bacc.Bacc("TRN2", target_bir_lowering=False, debug=False)

    x_ext = nc.dram_tensor("x", [S, H], F32R, kind="ExternalInput")
    xt_ext = nc.dram_tensor("xt", [H, S], F32R, kind="ExternalInput")
    w_ext = nc.dram_tensor("w", [H, H], F32R, kind="ExternalInput")
    o_ext = nc.dram_tensor("o", [S, NJ], F32R, kind="ExternalOutput")

    o_ap = o_ext.ap()
    # [S, H] -> [p, ki, h] super-tiles: 2 sequence tiles per 1 MiB DMA
    KI = 2                      # k-subtiles per super-tile
    KO = KS // KI               # 16 super-tiles
    x_r = x_ext.ap().rearrange("(ko ki p) h -> ko p ki h", p=P, ki=KI)
    w_r = w_ext.ap().rearrange("(kw p) h -> kw p h", p=P)
    # [H, S] -> [hi, ho, s] so a DMA grabs 128 h-partitions at once
    xt_r = xt_ext.ap().rearrange("(ho hi) s -> hi ho s", hi=P)
    SCC = 256                   # xt chunk width in s-columns (1 MiB)

    with tile.TileContext(nc) as tc:
        with (
            tc.tile_pool(name="stream", bufs=12) as stream_pool,
            tc.tile_pool(name="wk", bufs=8) as wk_pool,
            tc.tile_pool(name="gc", bufs=1) as gc_pool,
            tc.tile_pool(name="ot", bufs=6) as ot_pool,
            tc.tile_pool(name="ps", bufs=8, space="PSUM") as ps_pool,
        ):
            # PE warmup: dummy matmuls on a zero tile while the first x DMA
            # is in flight, so the HAM clock gate reaches 2.4 GHz before
            # real work starts (cold PE runs at 1.2 GHz for ~3.4 us)
            warm = gc_pool.tile([P, NJ + P], F32, name="warm")
            nc.vector.memset(warm[:, 0:8], 0.0)
            warm_r = warm[:].bitcast(F32R)
            warm_ps = ps_pool.tile([P, NJ], F32, tag="ps", name="warm_ps")
            for _ in range(8):
                nc.tensor.matmul(
                    warm_ps[:], warm_r[:, 0:P], warm_r[:, P : P + NJ], start=True, stop=True
                )

            # ---- pass 1: G[:, 0:512] = (x^T x)[:, 0:512] ----
            g_sb = gc_pool.tile([P, KH, NJ], F32R)
            g_ps = [ps_pool.tile([P, NJ], F32, tag="ps", name=f"g_ps{i}") for i in range(KH)]
            wks = []
            for ko in range(KO):
                if ko < 2:
                    # first supers split into half-DMAs so the first
                    # matmuls only wait on 512 KiB each
                    xs_halves = [
                        stream_pool.tile([P, 1, H], F32R, tag="head", bufs=4, name=f"xh{ko}_{i}")
                        for i in range(KI)
                    ]
                    for i in range(KI):
                        nc.sync.dma_start(xs_halves[i][:], x_r[ko, :, i : i + 1, :])
                else:
                    xs = stream_pool.tile([P, KI, H], F32R, tag="stream", name=f"xs{ko}")
                    nc.sync.dma_start(xs[:], x_r[ko])
                    xs_halves = None
                for ki in range(KI):
                    src = xs_halves[ki][:, 0] if xs_halves is not None else xs[:, ki]
                    for mi in range(KH):
                        nc.tensor.matmul(
                            g_ps[mi][:],
                            src[:, mi * P : (mi + 1) * P],
                            src[:, 0:NJ],
                            start=(ko == 0 and ki == 0),
                            stop=(ko == KO - 1 and ki == KI - 1),
                        )
                # spread the W prefetch through the back half of pass 1 so
                # it doesn't compete with the x stream at kernel start
                if ko >= KO - 8:
                    kw = ko - (KO - 8)
                    wk = wk_pool.tile([P, H], F32R, tag="wk", name=f"wk{kw}")
                    nc.sync.dma_start(wk[:], w_r[kw])
                    wks.append(wk)
            for mi in range(KH):
                nc.vector.tensor_copy(g_sb[:, mi, :], g_ps[mi][:])

            # ---- pass 2: C = W^T G ----
            c_sb = gc_pool.tile([P, KH, NJ], F32R)
            c_ps = [ps_pool.tile([P, NJ], F32, tag="ps", name=f"c_ps{i}") for i in range(KH)]
            for k2 in range(KH):
                for hi in range(KH):
                    nc.tensor.matmul(
                        c_ps[hi][:],
                        wks[k2][:, hi * P : (hi + 1) * P],
                        g_sb[:, k2, :],
                        start=(k2 == 0),
                        stop=(k2 == KH - 1),
                    )
            for hi in range(KH):
                nc.vector.tensor_copy(c_sb[:, hi, :], c_ps[hi][:])

            # ---- pass 3: out = x @ C  (x supplied transposed) ----
            # xt chunks share the stream pool slots, so their DMAs launch
            # exactly as pass-1 x tiles retire
            for sc in range(S // SCC):
                xt_c = stream_pool.tile([P, KH, SCC], F32R, tag="stream", name=f"xt{sc}")
                nc.sync.dma_start(xt_c[:], xt_r[:, :, sc * SCC : (sc + 1) * SCC])
                for ss in range(SCC // P):
                    o_ps = ps_pool.tile([P, NJ], F32, tag="ps")
                    for h in range(KH):
                        nc.tensor.matmul(
                            o_ps[:],
                            xt_c[:, h, ss * P : (ss + 1) * P],
                            c_sb[:, h, :],
                            start=(h == 0),
                            stop=(h == KH - 1),
                        )
                    o_t = ot_pool.tile([P, NJ], F32R, tag="ot")
                    nc.vector.tensor_copy(o_t[:], o_ps[:])
                    row = (sc * (SCC // P) + ss) * P
                    # outputs issue from the scalar engine (the other HWDGE
                    # ring) so their CAST-wait doesn't stall the xt prefetch
                    # stream on the sync engine
                    nc.scalar.dma_start(o_ap[row : row + P, :], o_t[:])

    nc.compile()
    return nc


def make_in_maps(hidden_states: np.ndarray, W_q: np.ndarray):
    """Shard full inputs into the 8 per-core input maps."""
    x = np.asarray(hidden_states, dtype=np.float32)
    w = np.asarray(W_q, dtype=np.float32)
    perms = [np.arange(H), np.r_[H // 2 : H, 0 : H // 2]]
    in_maps = []
    for c in range(N_CORES):
        b, j = c // 2, c % 2
        xb = np.ascontiguousarray(x[b])
        in_maps.append(
            {
                "x": np.ascontiguousarray(xb[:, perms[j]]),
                "xt": np.ascontiguousarray(xb.T),
                "w": np.ascontiguousarray(w[perms[j], :]),
            }
        )
    return in_maps


def run(hidden_states: np.ndarray, W_q: np.ndarray, **run_kwargs):
    """Build (cached), run on 8 cores, gather.  Returns (output, results)."""
    if "nc" not in _CACHE:
        _CACHE["nc"] = build_kernel()
    nc = _CACHE["nc"]
    in_maps = make_in_maps(hidden_states, W_q)
    res = run_bass_kernel_spmd(nc, in_maps, list(range(N_CORES)), **run_kwargs)
    B = N_CORES // 2
    out = np.empty((B, S, H), dtype=np.float32)
    for c in range(N_CORES):
        b, j = c // 2, c % 2
        out[b, :, j * NJ : (j + 1) * NJ] = res.results[c]["o"]
    return out, res


def kernel(hidden_states: np.ndarray, W_q: np.ndarray, **unused) -> np.ndarray:
    out, _ = run(hidden_states, W_q)
    return out


if __name__ == "__main__":
    rng = np.random.default_rng(0)
    x = rng.standard_normal((4, S, H), dtype=np.float32)
    w = (rng.standard_normal((H, H), dtype=np.float32) * 9.02e-5).astype(np.float32)
    out = kernel(hidden_states=x, W_q=w)
    xb = x[0].astype(np.float64)
    ref0 = (xb @ w.astype(np.float64).T) @ (xb.T @ xb) @ np.eye(H)  # sanity
    ref0 = (xb @ w.astype(np.float64).T @ (xb.T @ xb))
    err = np.abs(out[0] - ref0) / (np.abs(ref0).max() + 1e-30)
    print("max scale-relative err (batch 0):", err.max())


# revision 17
# speedup vs baseline: 1.0027x; 1.0027x over previous
"""Trainium2 Bass kernel for single-head dense attention without softmax.

Reference computation (B=4, S=4096, H=1024, fp32):
    q    = x @ W^T               [B, S, H]
    attn = (q @ x^T) @ x         [B, S, H]

There is no softmax, so the computation reorders to
    attn[b] = x[b] @ (W^T @ (x[b]^T @ x[b]))
which drops the FLOP count from ~309 GF to ~77 GF total.

Sharding over 8 NeuronCores: core c handles batch b = c//2 and output
columns jcols = [512*j, 512*j+512) with j = c%2.  Each core computes
    G = x[b]^T x[b]  restricted to columns jcols       (pass 1)
    C = W^T G[:, jcols]                                (pass 2)
    out[:, jcols] = x[b] @ C                           (pass 3)
To keep the device program identical across cores (SPMD), the host
permutes the H columns of x (and the H rows of W) per core so the
core's jcols always land in columns [0, 512).  Pass 3 consumes a
host-side transpose of x (fp32 has no DMA-transpose path on TRN2).
"""

import numpy as np

import concourse.bass as bass
import concourse.mybir as mybir
import concourse.tile as tile
from concourse import bacc
from concourse.bass_utils import run_bass_kernel_spmd

P = 128          # partitions / matmul contraction tile
S = 4096         # sequence length
H = 1024         # hidden
NJ = 512         # output columns per core
KS = S // P      # 32 sequence tiles
KH = H // P      # 8 hidden tiles
N_CORES = 8

F32R = mybir.dt.float32r
F32 = mybir.dt.float32

_CACHE: dict = {}


def build_kernel():
    nc = bacc.Bacc("TRN2", target_bir_lowering=False, debug=False)

    x_ext = nc.dram_tensor("x", [S, H], F32R, kind="ExternalInput")
    xt_ext = nc.dram_tensor("xt", [H, S], F32R, kind="ExternalInput")
    w_ext = nc.dram_tensor("w", [H, H], F32R, kind="ExternalInput")
    o_ext = nc.dram_tensor("o", [S, NJ], F32R, kind="ExternalOutput")

    o_ap = o_ext.ap()
    # [S, H] -> [p, ki, h] super-tiles: 2 sequence tiles per 1 MiB DMA
    KI = 2                      # k-subtiles per super-tile
    KO = KS // KI               # 16 super-tiles
    x_r = x_ext.ap().rearrange("(ko ki p) h -> ko p ki h", p=P, ki=KI)
    w_r = w_ext.ap().rearrange("(kw p) h -> kw p h", p=P)
    # [H, S] -> [hi, ho, s] so a DMA grabs 128 h-partitions at once
    xt_r = xt_ext.ap().rearrange("(ho hi) s -> hi ho s", hi=P)
    SCC = 256                   # xt chunk width in s-columns (1 MiB)

    with tile.TileContext(nc) as tc:
        with (
            tc.tile_pool(name="stream", bufs=12) as stream_pool,
            tc.tile_pool(name="wk", bufs=8) as wk_pool,
            tc.tile_pool(name="gc", bufs=1) as gc_pool,
            tc.tile_pool(name="ot", bufs=6) as ot_pool,
            tc.tile_pool(name="ps", bufs=8, space="PSUM") as ps_pool,
        ):
            # PE warmup: dummy matmuls on a zero tile while the first x DMA
            # is in flight, so the HAM clock gate reaches 2.4 GHz before
            # real work starts (cold PE runs at 1.2 GHz for ~3.4 us)
            warm = gc_pool.tile([P, NJ + P], F32, name="warm")
            nc.vector.memset(warm[:, 0:8], 0.0)
            warm_r = warm[:].bitcast(F32R)
            warm_ps = ps_pool.tile([P, NJ], F32, tag="ps", name="warm_ps")
            for _ in range(8):
                nc.tensor.matmul(
                    warm_ps[:], warm_r[:, 0:P], warm_r[:, P : P + NJ], start=True, stop=True
                )

            # ---- pass 1: G[:, 0:512] = (x^T x)[:, 0:512] ----
            g_sb = gc_pool.tile([P, KH, NJ], F32R)
            g_ps = [ps_pool.tile([P, NJ], F32, tag="ps", name=f"g_ps{i}") for i in range(KH)]
            wks = []
            for ko in range(KO):
                if ko == 0:
                    # first super split into two half-DMAs so the first
                    # matmul only waits on 512 KiB
                    xs_halves = [
                        stream_pool.tile([P, 1, H], F32R, tag="head", bufs=2, name=f"xh{i}")
                        for i in range(KI)
                    ]
                    for i in range(KI):
                        nc.sync.dma_start(xs_halves[i][:], x_r[0, :, i : i + 1, :])
                else:
                    xs = stream_pool.tile([P, KI, H], F32R, tag="stream", name=f"xs{ko}")
                    nc.sync.dma_start(xs[:], x_r[ko])
                    xs_halves = None
                for ki in range(KI):
                    src = xs_halves[ki][:, 0] if xs_halves is not None else xs[:, ki]
                    for mi in range(KH):
                        nc.tensor.matmul(
                            g_ps[mi][:],
                            src[:, mi * P : (mi + 1) * P],
                            src[:, 0:NJ],
                            start=(ko == 0 and ki == 0),
                            stop=(ko == KO - 1 and ki == KI - 1),
                        )
                # spread the W prefetch through the back half of pass 1 so
                # it doesn't compete with the x stream at kernel start
                if ko >= KO - 8:
                    kw = ko - (KO - 8)
                    wk = wk_pool.tile([P, H], F32R, tag="wk", name=f"wk{kw}")
                    nc.sync.dma_start(wk[:], w_r[kw])
                    wks.append(wk)
            for mi in range(KH):
                nc.vector.tensor_copy(g_sb[:, mi, :], g_ps[mi][:])

            # ---- pass 2: C = W^T G ----
            c_sb = gc_pool.tile([P, KH, NJ], F32R)
            c_ps = [ps_pool.tile([P, NJ], F32, tag="ps", name=f"c_ps{i}") for i in range(KH)]
            for k2 in range(KH):
                for hi in range(KH):
                    nc.tensor.matmul(
                        c_ps[hi][:],
                        wks[k2][:, hi * P : (hi + 1) * P],
                        g_sb[:, k2, :],
                        start=(k2 == 0),
                        stop=(k2 == KH - 1),
                    )
            for hi in range(KH):
                nc.vector.tensor_copy(c_sb[:, hi, :], c_ps[hi][:])

            # ---- pass 3: out = x @ C  (x supplied transposed) ----
            # xt chunks share the stream pool slots, so their DMAs launch
            # exactly as pass-1 x tiles retire
            for sc in range(S // SCC):
                xt_c = stream_pool.tile([P, KH, SCC], F32R, tag="stream", name=f"xt{sc}")
                nc.sync.dma_start(xt_c[:], xt_r[:, :, sc * SCC : (sc + 1) * SCC])
                for ss in range(SCC // P):
                    o_ps = ps_pool.tile([P, NJ], F32, tag="ps")
                    for h in range(KH):
                        nc.tensor.matmul(
                            o_ps[:],
                            xt_c[:, h, ss * P : (ss + 1) * P],
                            c_sb[:, h, :],
                            start=(h == 0),
                            stop=(h == KH - 1),
                        )
                    o_t = ot_pool.tile([P, NJ], F32R, tag="ot")
                    nc.vector.tensor_copy(o_t[:], o_ps[:])
                    row = (sc * (SCC // P) + ss) * P
                    # outputs issue from the scalar engine (the other HWDGE
                    # ring) so their CAST-wait doesn't stall the xt prefetch
                    # stream on the sync engine
                    nc.scalar.dma_start(o_ap[row : row + P, :], o_t[:])

    nc.compile()
    return nc


def make_in_maps(hidden_states: np.ndarray, W_q: np.ndarray):
    """Shard full inputs into the 8 per-core input maps."""
    x = np.asarray(hidden_states, dtype=np.float32)
    w = np.asarray(W_q, dtype=np.float32)
    perms = [np.arange(H), np.r_[H // 2 : H, 0 : H // 2]]
    in_maps = []
    for c in range(N_CORES):
        b, j = c // 2, c % 2
        xb = np.ascontiguousarray(x[b])
        in_maps.append(
            {
                "x": np.ascontiguousarray(xb[:, perms[j]]),
                "xt": np.ascontiguousarray(xb.T),
                "w": np.ascontiguousarray(w[perms[j], :]),
            }
        )
    return in_maps


def run(hidden_states: np.ndarray, W_q: np.ndarray, **run_kwargs):
    """Build (cached), run on 8 cores, gather.  Returns (output, results)."""
    if "nc" not in _CACHE:
        _CACHE["nc"] = build_kernel()
    nc = _CACHE["nc"]
    in_maps = make_in_maps(hidden_states, W_q)
    res = run_bass_kernel_spmd(nc, in_maps, list(range(N_CORES)), **run_kwargs)
    B = N_CORES // 2
    out = np.empty((B, S, H), dtype=np.float32)
    for c in range(N_CORES):
        b, j = c // 2, c % 2
        out[b, :, j * NJ : (j + 1) * NJ] = res.results[c]["o"]
    return out, res


def kernel(hidden_states: np.ndarray, W_q: np.ndarray, **unused) -> np.ndarray:
    out, _ = run(hidden_states, W_q)
    return out


if __name__ == "__main__":
    rng = np.random.default_rng(0)
    x = rng.standard_normal((4, S, H), dtype=np.float32)
    w = (rng.standard_normal((H, H), dtype=np.float32) * 9.02e-5).astype(np.float32)
    out = kernel(hidden_states=x, W_q=w)
    xb = x[0].astype(np.float64)
    ref0 = (xb @ w.astype(np.float64).T) @ (xb.T @ xb) @ np.eye(H)  # sanity
    ref0 = (xb @ w.astype(np.float64).T @ (xb.T @ xb))
    err = np.abs(out[0] - ref0) / (np.abs(ref0).max() + 1e-30)
    print("max scale-relative err (batch 0):", err.max())
